# revision 23
# baseline (speedup 1.0000x reference)
# Self-contained Trainium2 Bass kernel for the 2-layer dense GAT problem.
# kernel(**inputs) takes FULL inputs, shards rows across 8 NeuronCores,
# runs one SPMD Bass program (with mid-kernel AllGathers), returns FULL output.
#
# Layer-1/2 attention strategy: exp(leakyrelu(s)) == max(exp(s), exp(a*s))
# exactly for a<=1, and exp(src_i+dst_j) factors rank-1.  So the masked
# attention matrix e2 = adj * max(exp(s), exp(a*s)) is built per [128j,512i]
# tile with two scalar-engine Exp activations (per-partition dst bias) or two
# DVE tensor_scalar rank-1 products, a max and an adj-mask multiply, spread
# across Scalar/Vector/GpSimd.  Aggregation is a single transposed matmul
# (stationary = Whaug[j,65]) per tile, accumulating [65,512] in PSUM, giving
# h1T [feat, i] directly (no transpose phase).
import os
import sys

os.environ.setdefault("JAX_PLATFORMS", "axon")
sys.path.insert(0, "/opt/trn_rl_repo")

import numpy as np
import ml_dtypes

import concourse.bass as bass
import concourse.bacc as bacc
import concourse.tile as tile
from concourse import mybir

BF16 = ml_dtypes.bfloat16
F32 = mybir.dt.float32
BF = mybir.dt.bfloat16

N, FEAT, H, O, OUT = 4096, 512, 8, 64, 512
NC = 8
RB = N // NC          # 512 rows per core
JT = N // 128         # 32 j tiles
KT = FEAT // 128      # 4 feature tiles
IT = RB // 128        # 4 local row tiles
ALPHA = 0.2

AL = mybir.AluOpType
AF = mybir.ActivationFunctionType

# head formulation mix (see module docstring); balances Scalar vs Vector load
D_HEADS = (0, 1, 2, 3)        # both exps on scalar engine
DR_HEADS = (4,)               # exp(s) on scalar, exp(a*s) on DVE
R_HEADS = (5, 6, 7)           # both exps as DVE rank-1 products
JG = 4                        # j-tiles per batched DVE max/mask op


def _bc(ap, n):
    """Broadcast an AP along a new innermost free dim of size n (step 0)."""
    return bass.AP(tensor=ap.tensor, offset=ap.offset, ap=list(ap.ap) + [[0, n]])


def build_program():
    nc = bacc.Bacc("TRN2", target_bir_lowering=False, debug=False, num_devices=NC)

    # ---------------- DRAM I/O ----------------
    d_xT = nc.dram_tensor("xT", [FEAT, N], BF, kind="ExternalInput")          # replicated
    d_xlocT = nc.dram_tensor("xlocT", [FEAT, RB], BF, kind="ExternalInput")   # per-core
    d_a01 = nc.dram_tensor("a01T", [N, RB], BF, kind="ExternalInput")         # per-core
    d_w1 = nc.dram_tensor("w1all", [FEAT, H * O], BF, kind="ExternalInput")
    d_waux = nc.dram_tensor("wauxall", [FEAT, 16], BF, kind="ExternalInput")  # [dst8|src8]
    d_wo = nc.dram_tensor("woall", [FEAT, OUT], BF, kind="ExternalInput")
    d_w2aux = nc.dram_tensor("w2aux", [FEAT, 2], BF, kind="ExternalInput")    # [dst|src]
    d_l1w = nc.dram_tensor("l1w", [OUT, 1024], BF, kind="ExternalInput")
    d_l1b = nc.dram_tensor("l1b", [1, 1024], BF, kind="ExternalInput")
    d_l2w = nc.dram_tensor("l2w", [1024, OUT], BF, kind="ExternalInput")
    d_l2b = nc.dram_tensor("l2b", [1, OUT], BF, kind="ExternalInput")
    d_out = nc.dram_tensor("out", [RB, OUT], F32, kind="ExternalOutput")

    # internal DRAM scratch
    d_srcrow = nc.dram_tensor("srcrow_d", [H, RB], BF)
    d_esrow = nc.dram_tensor("esrow_d", [H, RB], BF)
    d_esarow = nc.dram_tensor("esarow_d", [H, RB], BF)
    d_s2row = nc.dram_tensor("s2row_d", [1, RB], BF)
    d_cc2 = nc.dram_tensor("cc2_d", [RB, 513], BF)
    d_cc2o0 = nc.dram_tensor("cc2o0_d", [N // 2, 513], BF, addr_space="Shared")
    d_cc2o1 = nc.dram_tensor("cc2o1_d", [N // 2, 513], BF, addr_space="Shared")
    d_cc2a = nc.dram_tensor("cc2a_d", [RB, 1], BF)
    d_cc2ao = nc.dram_tensor("cc2ao_d", [N, 1], BF, addr_space="Shared")

    with tile.TileContext(nc) as tc:
        import contextlib
        ctx = contextlib.ExitStack()
        with ctx:
            consts = ctx.enter_context(tc.tile_pool(name="consts", bufs=1))

            ones_row = consts.tile([1, RB], BF)
            nc.vector.memset(ones_row, 1.0)
            ones64 = consts.tile([1, O], BF)
            nc.vector.memset(ones64, 1.0)

            # ---------------- weights into SBUF (phase-W inputs first) ----
            w1_sb = consts.tile([128, KT, H * O], BF)
            nc.sync.dma_start(w1_sb, d_w1.ap().rearrange("(k p) o -> p k o", p=128))
            waux_sb = consts.tile([128, KT, 16], BF)
            nc.sync.dma_start(waux_sb, d_waux.ap().rearrange("(k p) o -> p k o", p=128))

            p_a01 = ctx.enter_context(tc.tile_pool(name="p_a01", bufs=1))
            a01_sb = p_a01.tile([128, JT, RB], BF)
            p_h1 = ctx.enter_context(tc.tile_pool(name="p_h1", bufs=1))
            h1T = p_h1.tile([128, IT, RB], BF)   # layer-1 out, [feat, local-row]

            import contextlib as _cl
            l1ctx = _cl.ExitStack()
            big1 = l1ctx.enter_context(tc.tile_pool(name="big1", bufs=1))

            # ---------------- Phase W: Wh1 + aux projections (full N) -----
            whaug = big1.tile([128, JT, H, O + 1], BF)   # per-j Wh per head + ones col
            sdvec = big1.tile([128, JT, 16], F32)        # dst/src projections, all j
            srcrow_sb = consts.tile([H, RB], F32)

            with tc.tile_pool(name="xt_pool", bufs=1) as xtp, \
                 tc.tile_pool(name="wps", bufs=2, space="PSUM") as wps:
                xT_sb = xtp.tile([128, KT, N], BF)
                nc.sync.dma_start(xT_sb, d_xT.ap().rearrange("(k p) n -> p k n", p=128))
                xlocT_sb = xtp.tile([128, KT, RB], BF)
                nc.sync.dma_start(xlocT_sb, d_xlocT.ap().rearrange("(k p) i -> p k i", p=128))

                # mask next (phase A), late weights after
                nc.sync.dma_start(a01_sb, d_a01.ap().rearrange("(t p) i -> p t i", p=128))
                wo_sb = consts.tile([128, KT, OUT], BF)
                nc.sync.dma_start(wo_sb, d_wo.ap().rearrange("(k p) o -> p k o", p=128))
                w2aux_sb = consts.tile([128, KT, 2], BF)
                nc.sync.dma_start(w2aux_sb, d_w2aux.ap().rearrange("(k p) o -> p k o", p=128))
                l1w_sb = consts.tile([128, KT, 1024], BF)
                nc.sync.dma_start(l1w_sb, d_l1w.ap().rearrange("(k p) o -> p k o", p=128))
                l1b_sb = consts.tile([1, 1024], BF)
                nc.sync.dma_start(l1b_sb, d_l1b.ap())
                l2w_sb = consts.tile([128, 8, OUT], BF)
                nc.sync.dma_start(l2w_sb, d_l2w.ap().rearrange("(k p) o -> p k o", p=128))
                l2b_sb = consts.tile([1, OUT], BF)
                nc.sync.dma_start(l2b_sb, d_l2b.ap())

                for jt in range(JT):
                    ps_wh = wps.tile([128, H * O], F32)
                    ps_aux = wps.tile([128, 16], F32, tag="aux")
                    for kt in range(KT):
                        lhsT = xT_sb[:, kt, jt * 128:(jt + 1) * 128]
                        nc.tensor.matmul(ps_wh, lhsT, w1_sb[:, kt, :],
                                         start=(kt == 0), stop=(kt == KT - 1))
                        nc.tensor.matmul(ps_aux, lhsT, waux_sb[:, kt, :],
                                         start=(kt == 0), stop=(kt == KT - 1))
                    nc.scalar.copy(whaug[:, jt, :, 0:O],
                                   ps_wh.rearrange("p (h o) -> p h o", h=H))
                    nc.vector.tensor_copy(sdvec[:, jt, :], ps_aux)

                # src of local rows as rows: [H, RB] = w_src.T @ xloc
                ps_row = wps.tile([H, RB], F32, tag="aux")
                for kt in range(KT):
                    nc.tensor.matmul(ps_row, waux_sb[:, kt, 8:16], xlocT_sb[:, kt, :],
                                     start=(kt == 0), stop=(kt == KT - 1))
                nc.vector.tensor_copy(srcrow_sb, ps_row)

            nc.vector.memset(whaug[:, :, :, O], 1.0)  # ones (denominator) column

            # per-partition dst scalars: alpha-scaled copy (f32 for act bias)
            sdveca = big1.tile([128, JT, H], F32)
            nc.vector.tensor_scalar(sdveca, sdvec[:, :, 0:H], ALPHA, None, op0=AL.mult)
            # R-heads: exp(dst), exp(alpha*dst) per-partition scalars
            edv = big1.tile([128, JT, H], F32)
            nc.scalar.activation(edv, sdvec[:, :, 0:H], AF.Exp)
            edva = big1.tile([128, JT, H], F32)
            nc.scalar.activation(edva, sdvec[:, :, 0:H], AF.Exp, scale=ALPHA)

            # src rows -> DRAM -> partition-broadcast tiles
            srcrow_bf = consts.tile([H, RB], BF)
            nc.vector.tensor_copy(srcrow_bf, srcrow_sb)
            nc.sync.dma_start(d_srcrow.ap(), srcrow_bf)
            esrow_sb = consts.tile([H, RB], BF)
            nc.scalar.activation(esrow_sb, srcrow_sb, AF.Exp)
            nc.sync.dma_start(d_esrow.ap(), esrow_sb)
            esarow_sb = consts.tile([H, RB], BF)
            nc.scalar.activation(esarow_sb, srcrow_sb, AF.Exp, scale=ALPHA)
            nc.sync.dma_start(d_esarow.ap(), esarow_sb)

            srcb = big1.tile([128, H, RB], BF)
            for h in D_HEADS + DR_HEADS:
                nc.sync.dma_start(srcb[:, h, :],
                                  d_srcrow.ap()[h:h + 1, :].to_broadcast([128, RB]))
            esb = big1.tile([128, len(R_HEADS), RB], BF)
            for k, h in enumerate(R_HEADS):
                nc.sync.dma_start(esb[:, k, :],
                                  d_esrow.ap()[h:h + 1, :].to_broadcast([128, RB]))
            esab = big1.tile([128, len(R_HEADS) + len(DR_HEADS), RB], BF)
            for k, h in enumerate(R_HEADS + DR_HEADS):
                nc.sync.dma_start(esab[:, k, :],
                                  d_esarow.ap()[h:h + 1, :].to_broadcast([128, RB]))

            # ---------------- Phase A: layer-1 attention ------------------
            with tc.tile_pool(name="e2p", bufs=4) as e2p, \
                 tc.tile_pool(name="aps", bufs=2, space="PSUM") as aps, \
                 tc.tile_pool(name="bps", bufs=2, space="PSUM") as bps, \
                 tc.tile_pool(name="comb", bufs=2) as comb:
                for h in range(H):
                    psH = aps.tile([O + 1, RB], F32, tag="psH", name=f"psH{h}")
                    for jg in range(JT // JG):
                        a1g = e2p.tile([128, JG, RB], BF, tag="a1")
                        a2g = e2p.tile([128, JG, RB], BF, tag="a2")
                        for q in range(JG):
                            jt = jg * JG + q
                            if h in D_HEADS or h in DR_HEADS:
                                nc.scalar.activation(a1g[:, q, :], srcb[:, h, :],
                                                     AF.Exp,
                                                     bias=sdvec[:, jt, h:h + 1])
                            else:
                                k = R_HEADS.index(h)
                                nc.vector.tensor_scalar(a1g[:, q, :], esb[:, k, :],
                                                        edv[:, jt, h:h + 1], None,
                                                        op0=AL.mult)
                            if h in D_HEADS:
                                nc.scalar.activation(a2g[:, q, :], srcb[:, h, :],
                                                     AF.Exp,
                                                     bias=sdveca[:, jt, h:h + 1],
                                                     scale=ALPHA)
                            else:
                                k = (R_HEADS + DR_HEADS).index(h)
                                nc.vector.tensor_scalar(a2g[:, q, :], esab[:, k, :],
                                                        edva[:, jt, h:h + 1], None,
                                                        op0=AL.mult)
                        # batched max + adj-mask over the 4-tile group
                        e2g = e2p.tile([128, JG, RB], BF, tag="e2")
                        nc.vector.tensor_tensor(a1g, a1g, a2g, op=AL.max)
                        nc.vector.tensor_tensor(
                            e2g, a1g, a01_sb[:, jg * JG:(jg + 1) * JG, :], op=AL.mult)
                        for q in range(JG):
                            jt = jg * JG + q
                            nc.tensor.matmul(psH, whaug[:, jt, h, :], e2g[:, q, :],
                                             start=(jt == 0), stop=(jt == JT - 1))

                    # combine: h1 = elu(num/den), written transposed into h1T.
                    # 1/den = exp(-ln(den)) on the scalar engine: the DVE
                    # reciprocal is an 8x iterative op (3.3us on [1,512]).
                    lden = comb.tile([1, RB], F32, tag="lden")
                    nc.scalar.activation(lden, psH[O:O + 1, :], AF.Ln)
                    rec = comb.tile([1, RB], BF, tag="rec")
                    nc.scalar.activation(rec, lden, AF.Exp, scale=-1.0)
                    psB = bps.tile([O, RB], F32)
                    nc.tensor.matmul(psB, ones64, rec)
                    nsb = comb.tile([O, RB], F32, tag="nsb")
                    nc.vector.tensor_copy(nsb, psH[0:O, :])
                    h1r = comb.tile([O, RB], F32, tag="h1r")
                    nc.vector.tensor_tensor(h1r, nsb, psB, op=AL.mult)
                    ex = comb.tile([O, RB], F32, tag="ex")
                    nc.scalar.activation(ex, h1r, AF.Exp)
                    nc.vector.tensor_scalar(ex, ex, 1.0, -1.0, op0=AL.min, op1=AL.add)
                    p0 = (h % 2) * O
                    nc.vector.scalar_tensor_tensor(
                        h1T[p0:p0 + O, h // 2, :], in0=h1r, scalar=0.0, in1=ex,
                        op0=AL.max, op1=AL.add)

            l1ctx.close()
            big2 = ctx.enter_context(tc.tile_pool(name="big2", bufs=1))

            # ---------------- Phase W2: local Wh2 + AllGather -------------
            cc2s = big2.tile([128, IT, 514], BF)   # 514 keeps 4B alignment
            with tc.tile_pool(name="w2ps", bufs=2, space="PSUM") as w2ps:
                for it in range(IT):
                    ps2 = w2ps.tile([128, OUT], F32)
                    ps2a = w2ps.tile([128, 2], F32, tag="aux2")
                    for kt in range(KT):
                        lhsT = h1T[:, kt, it * 128:(it + 1) * 128]
                        nc.tensor.matmul(ps2, lhsT, wo_sb[:, kt, :],
                                         start=(kt == 0), stop=(kt == KT - 1))
                        nc.tensor.matmul(ps2a, lhsT, w2aux_sb[:, kt, :],
                                         start=(kt == 0), stop=(kt == KT - 1))
                    nc.scalar.copy(cc2s[:, it, 0:OUT], ps2)
                    nc.vector.tensor_copy(cc2s[:, it, OUT:OUT + 1], ps2a[:, 0:1])

                # local src2 row
                ps_r2 = w2ps.tile([1, RB], F32, tag="aux2")
                for kt in range(KT):
                    nc.tensor.matmul(ps_r2, w2aux_sb[:, kt, 1:2], h1T[:, kt, :],
                                     start=(kt == 0), stop=(kt == KT - 1))
                s2row_sb = consts.tile([1, RB], BF)
                nc.vector.tensor_copy(s2row_sb, ps_r2)
                nc.sync.dma_start(d_s2row.ap(), s2row_sb)

            nc.sync.dma_start(d_cc2a.ap().rearrange("(t p) c -> p t c", p=128),
                              cc2s[:, :, OUT:OUT + 1])
            nc.gpsimd.collective_compute(
                "AllGather", AL.bypass, ins=[d_cc2a.ap().opt()],
                outs=[d_cc2ao.ap().opt()], replica_groups=[list(range(NC))])
            # big gather in 2 row-chunks so A2 matmuls start on chunk 0
            for cg in range(2):
                nc.sync.dma_start(
                    d_cc2.ap()[cg * 256:(cg + 1) * 256, :]
                    .rearrange("(t p) c -> p t c", p=128),
                    cc2s[:, cg * 2:(cg + 1) * 2, 0:513])
                nc.gpsimd.collective_compute(
                    "AllGather", AL.bypass,
                    ins=[d_cc2.ap()[cg * 256:(cg + 1) * 256, :].opt()],
                    outs=[(d_cc2o0 if cg == 0 else d_cc2o1).ap().opt()],
                    replica_groups=[list(range(NC))])

            # dst2 per-partition scalars (from mini-gather) + src2 broadcast
            dst2b = big2.tile([128, JT, 1], BF)
            nc.sync.dma_start(dst2b, d_cc2ao.ap().rearrange("(t p) c -> p t c", p=128))
            dst2v = big2.tile([128, JT, 1], F32)
            nc.vector.tensor_copy(dst2v, dst2b)
            dst2va = big2.tile([128, JT, 1], F32)
            nc.vector.tensor_scalar(dst2va, dst2b, ALPHA, None, op0=AL.mult)
            src2b = big2.tile([128, RB], BF)
            nc.sync.dma_start(src2b, d_s2row.ap()[0:1, :].to_broadcast([128, RB]))

            # gathered Wh2 chunks: [p, chunk, (core,itm), 514]; col 512 = dst2
            # (unused), col 513 = ones (denominator row)
            wh2aug = big2.tile([128, 2, 16, 514], BF)
            nc.sync.dma_start(wh2aug[:, 0, :, 0:513],
                              d_cc2o0.ap().rearrange("(t p) c -> p t c", p=128))
            nc.sync.dma_start(wh2aug[:, 1, :, 0:513],
                              d_cc2o1.ap().rearrange("(t p) c -> p t c", p=128))
            nc.vector.memset(wh2aug[:, :, :, 513], 1.0)

            # ---------------- Phase A2: layer-2 attention (transposed) ----
            # e2 tiles for all jt are produced up-front (overlaps the gather);
            # matmuls consume them in gather-chunk order.
            o2T = big2.tile([128, IT, RB], BF)
            e2all = big2.tile([128, JT, RB], BF)
            with tc.tile_pool(name="e22p", bufs=2) as e22p, \
                 tc.tile_pool(name="a2ps", bufs=1, space="PSUM") as a2ps, \
                 tc.tile_pool(name="b2ps", bufs=2, space="PSUM") as b2ps, \
                 tc.tile_pool(name="c2", bufs=2) as c2:
                for jg in range(JT // JG):
                    a1g = e22p.tile([128, JG, RB], BF, tag="a1")
                    a2g = e22p.tile([128, JG, RB], BF, tag="a2")
                    for q in range(JG):
                        jt = jg * JG + q
                        nc.scalar.activation(a1g[:, q, :], src2b, AF.Exp,
                                             bias=dst2v[:, jt, 0:1])
                        nc.scalar.activation(a2g[:, q, :], src2b, AF.Exp,
                                             bias=dst2va[:, jt, 0:1], scale=ALPHA)
                    nc.vector.tensor_tensor(a1g, a1g, a2g, op=AL.max)
                    nc.vector.tensor_tensor(
                        e2all[:, jg * JG:(jg + 1) * JG, :], a1g,
                        a01_sb[:, jg * JG:(jg + 1) * JG, :], op=AL.mult)

                psO = [a2ps.tile([128, RB], F32, tag=f"psO{c}", name=f"psO{c}")
                       for c in range(IT)]
                psD = a2ps.tile([1, RB], F32, tag="psD", name="psD")
                for cg in range(2):
                    for u in range(16):
                        jt = (u // 2) * 4 + cg * 2 + (u % 2)
                        st = (cg == 0 and u == 0)
                        sp = (cg == 1 and u == 15)
                        e2 = e2all[:, jt, :]
                        for c in range(IT):
                            nc.tensor.matmul(psO[c],
                                             wh2aug[:, cg, u, c * 128:(c + 1) * 128],
                                             e2, start=st, stop=sp)
                        nc.tensor.matmul(psD, wh2aug[:, cg, u, 513:514], e2,
                                         start=st, stop=sp)

                lden2 = c2.tile([1, RB], F32, tag="lden2")
                nc.scalar.activation(lden2, psD, AF.Ln)
                rec2 = c2.tile([1, RB], BF, tag="rec2")
                nc.scalar.activation(rec2, lden2, AF.Exp, scale=-1.0)
                psB2 = b2ps.tile([128, RB], F32)
                ones128 = consts.tile([1, 128], BF)
                nc.vector.memset(ones128, 1.0)
                nc.tensor.matmul(psB2, ones128, rec2)
                for c in range(IT):
                    nsb2 = c2.tile([128, RB], F32, tag="nsb2")
                    nc.scalar.copy(nsb2, psO[c])
                    o2r = c2.tile([128, RB], F32, tag="o2r")
                    nc.vector.tensor_tensor(o2r, nsb2, psB2, op=AL.mult)
                    ex2 = c2.tile([128, RB], F32, tag="ex2")
                    nc.scalar.activation(ex2, o2r, AF.Exp)
                    nc.vector.tensor_scalar(ex2, ex2, 1.0, -1.0, op0=AL.min, op1=AL.add)
                    nc.vector.scalar_tensor_tensor(o2T[:, c, :], in0=o2r, scalar=0.0,
                                                   in1=ex2, op0=AL.max, op1=AL.add)

            # ---------------- lin1 + lin2 ---------------------------------
            o3T = big2.tile([128, 8, RB], BF)
            out_sb = big2.tile([128, IT, OUT], F32)
            with tc.tile_pool(name="l_ps", bufs=4, space="PSUM") as lps:
                for mq in range(8):
                    ps = lps.tile([128, RB], F32)
                    for kt in range(KT):
                        nc.tensor.matmul(ps, l1w_sb[:, kt, mq * 128:(mq + 1) * 128],
                                         o2T[:, kt, :], start=(kt == 0), stop=False)
                    nc.tensor.matmul(ps, l1b_sb[:, mq * 128:(mq + 1) * 128], ones_row,
                                     start=False, stop=True)
                    nc.scalar.activation(o3T[:, mq, :], ps, AF.Relu)

                for mi in range(IT):
                    ps = lps.tile([128, OUT], F32)
                    for kq in range(8):
                        nc.tensor.matmul(ps, o3T[:, kq, mi * 128:(mi + 1) * 128],
                                         l2w_sb[:, kq, :], start=(kq == 0), stop=False)
                    nc.tensor.matmul(ps, ones_row[:, 0:128], l2b_sb,
                                     start=False, stop=True)
                    nc.vector.tensor_copy(out_sb[:, mi, :], ps)

            nc.sync.dma_start(d_out.ap().rearrange("(t p) o -> p t o", p=128), out_sb)

    nc.compile()
    return nc


_CACHE = {}


def _prep_inputs(inputs):
    x = np.asarray(inputs["x"], np.float32)
    adj = np.asarray(inputs["adj"])
    W1 = np.asarray(inputs["W1"], np.float32)
    a1 = np.asarray(inputs["a1"], np.float32)
    Wo = np.asarray(inputs["Wo"], np.float32)
    ao = np.asarray(inputs["ao"], np.float32)
    l1w = np.asarray(inputs["lin1_w"], np.float32)
    l1b = np.asarray(inputs["lin1_b"], np.float32)
    l2w = np.asarray(inputs["lin2_w"], np.float32)
    l2b = np.asarray(inputs["lin2_b"], np.float32)

    xT = np.ascontiguousarray(x.T).astype(BF16)
    w_dst = np.einsum("hfo,ho->fh", W1, a1[:, O:]).astype(np.float32)   # [F, H]
    w_src = np.einsum("hfo,ho->fh", W1, a1[:, :O]).astype(np.float32)
    wauxall = np.ascontiguousarray(
        np.concatenate([w_dst, w_src], axis=1)).astype(BF16)            # [F, 16]
    w1all = np.ascontiguousarray(W1.transpose(1, 0, 2).reshape(FEAT, H * O)).astype(BF16)
    w2aux = np.ascontiguousarray(
        np.stack([Wo @ ao[OUT:], Wo @ ao[:OUT]], axis=1)).astype(BF16)  # [F, 2]

    rep = dict(
        xT=xT, w1all=w1all, wauxall=wauxall, woall=np.ascontiguousarray(Wo).astype(BF16),
        w2aux=w2aux,
        l1w=np.ascontiguousarray(l1w).astype(BF16),
        l1b=np.ascontiguousarray(l1b.reshape(1, -1)).astype(BF16),
        l2w=np.ascontiguousarray(l2w).astype(BF16),
        l2b=np.ascontiguousarray(l2b.reshape(1, -1)).astype(BF16),
    )
    in_maps = []
    for c in range(NC):
        rows = slice(c * RB, (c + 1) * RB)
        m = dict(rep)
        m["xlocT"] = np.ascontiguousarray(x[rows, :].T).astype(BF16)
        m["a01T"] = np.ascontiguousarray((adj[rows, :] > 0).T.astype(BF16))
        in_maps.append(m)
    return in_maps


def kernel(**inputs):
    from concourse.bass_utils import run_bass_kernel_spmd

    if "nc" not in _CACHE:
        _CACHE["nc"] = build_program()
    nc = _CACHE["nc"]

    in_maps = _prep_inputs(inputs)
    trace = bool(_CACHE.get("trace"))
    res = run_bass_kernel_spmd(nc, in_maps, core_ids=list(range(NC)), trace=trace)
    _CACHE["last_results"] = res
    out = np.concatenate([r["out"] for r in res.results], axis=0)
    return out.astype(np.float32)


# revision 36
# speedup vs baseline: 1.2034x; 1.2034x over previous
# Self-contained Trainium2 Bass kernel for the 2-layer dense GAT problem.
# kernel(**inputs) takes FULL inputs, shards rows across 8 NeuronCores,
# runs one SPMD Bass program (with mid-kernel AllGathers), returns FULL output.
#
# Layer-1/2 attention strategy: exp(leakyrelu(s)) == max(exp(s), exp(a*s))
# exactly for a<=1, and exp(src_i+dst_j) factors rank-1.  So the masked
# attention matrix e2 = adj * max(exp(s), exp(a*s)) is built per [128j,512i]
# tile with two scalar-engine Exp activations (per-partition dst bias) or two
# DVE tensor_scalar rank-1 products, a max and an adj-mask multiply, spread
# across Scalar/Vector/GpSimd.  Aggregation is a single transposed matmul
# (stationary = Whaug[j,65]) per tile, accumulating [65,512] in PSUM, giving
# h1T [feat, i] directly (no transpose phase).
import os
import sys

os.environ.setdefault("JAX_PLATFORMS", "axon")
sys.path.insert(0, "/opt/trn_rl_repo")

import numpy as np
import ml_dtypes

import concourse.bass as bass
import concourse.bacc as bacc
import concourse.tile as tile
from concourse import mybir

BF16 = ml_dtypes.bfloat16
F32 = mybir.dt.float32
BF = mybir.dt.bfloat16

N, FEAT, H, O, OUT = 4096, 512, 8, 64, 512
NC = 8
RB = N // NC          # 512 rows per core
JT = N // 128         # 32 j tiles
KT = FEAT // 128      # 4 feature tiles
IT = RB // 128        # 4 local row tiles
ALPHA = 0.2

AL = mybir.AluOpType
AF = mybir.ActivationFunctionType

# head formulation mix (see module docstring); balances Scalar vs Vector load
D_HEADS = (0, 1, 2, 3)        # both exps on scalar engine
DR_HEADS = (4,)               # exp(s) on scalar, exp(a*s) on DVE
R_HEADS = (5, 6, 7)           # both exps as DVE rank-1 products
JG = 4                        # j-tiles per batched DVE max/mask op


def _bc(ap, n):
    """Broadcast an AP along a new innermost free dim of size n (step 0)."""
    return bass.AP(tensor=ap.tensor, offset=ap.offset, ap=list(ap.ap) + [[0, n]])


def build_program():
    nc = bacc.Bacc("TRN2", target_bir_lowering=False, debug=False, num_devices=NC)

    # ---------------- DRAM I/O ----------------
    d_xT = nc.dram_tensor("xT", [FEAT, N], BF, kind="ExternalInput")          # replicated
    d_xlocT = nc.dram_tensor("xlocT", [FEAT, RB], BF, kind="ExternalInput")   # per-core
    d_a01 = nc.dram_tensor("a01T", [N, RB], BF, kind="ExternalInput")         # per-core
    d_w1 = nc.dram_tensor("w1all", [FEAT, H * O], BF, kind="ExternalInput")
    d_waux = nc.dram_tensor("wauxall", [FEAT, 16], BF, kind="ExternalInput")  # [dst8|src8]
    d_wo = nc.dram_tensor("woall", [FEAT, OUT], BF, kind="ExternalInput")
    d_w2aux = nc.dram_tensor("w2aux", [FEAT, 2], BF, kind="ExternalInput")    # [dst|src]
    d_l1w = nc.dram_tensor("l1w", [OUT, 1024], BF, kind="ExternalInput")
    d_l1b = nc.dram_tensor("l1b", [1, 1024], BF, kind="ExternalInput")
    d_l2w = nc.dram_tensor("l2w", [1024, OUT], BF, kind="ExternalInput")
    d_l2b = nc.dram_tensor("l2b", [1, OUT], BF, kind="ExternalInput")
    d_out = nc.dram_tensor("out", [RB, OUT], F32, kind="ExternalOutput")

    # internal DRAM scratch
    d_srcrow = nc.dram_tensor("srcrow_d", [H, RB], BF)
    d_recs = nc.dram_tensor("recs_d", [H, RB], BF)
    d_esrow = nc.dram_tensor("esrow_d", [H, RB], BF)
    d_esarow = nc.dram_tensor("esarow_d", [H, RB], BF)
    d_s2row = nc.dram_tensor("s2row_d", [1, RB], BF)
    d_es2row = nc.dram_tensor("es2row_d", [1, RB], BF)
    d_esa2row = nc.dram_tensor("esa2row_d", [1, RB], BF)
    d_cc2 = nc.dram_tensor("cc2_d", [RB, 513], BF)
    d_cc2o0 = nc.dram_tensor("cc2o0_d", [N // 2, 513], BF, addr_space="Shared")
    d_cc2o1 = nc.dram_tensor("cc2o1_d", [N // 2, 513], BF, addr_space="Shared")
    d_cc2a = nc.dram_tensor("cc2a_d", [RB, 1], BF)
    d_cc2ao = nc.dram_tensor("cc2ao_d", [N, 1], BF, addr_space="Shared")

    with tile.TileContext(nc) as tc:
        import contextlib
        ctx = contextlib.ExitStack()
        with ctx:
            consts = ctx.enter_context(tc.tile_pool(name="consts", bufs=1))

            ones_row = consts.tile([1, RB], BF)
            nc.vector.memset(ones_row, 1.0)
            ones64 = consts.tile([1, O], BF)
            nc.vector.memset(ones64, 1.0)

            # ---------------- weights into SBUF (phase-W inputs first) ----
            w1_sb = consts.tile([128, KT, H * O], BF)
            nc.sync.dma_start(w1_sb, d_w1.ap().rearrange("(k p) o -> p k o", p=128))
            waux_sb = consts.tile([128, KT, 16], BF)
            nc.sync.dma_start(waux_sb, d_waux.ap().rearrange("(k p) o -> p k o", p=128))

            p_a01 = ctx.enter_context(tc.tile_pool(name="p_a01", bufs=1))
            a01_sb = p_a01.tile([128, JT, RB], BF)
            p_h1 = ctx.enter_context(tc.tile_pool(name="p_h1", bufs=1))
            h1T = p_h1.tile([128, IT, RB], BF)   # layer-1 out, [feat, local-row]

            import contextlib as _cl
            l1ctx = _cl.ExitStack()
            big1 = l1ctx.enter_context(tc.tile_pool(name="big1", bufs=1))

            # ---------------- Phase W: Wh1 + aux projections (full N) -----
            whaug = big1.tile([128, JT, H, O + 1], BF)   # per-j Wh per head + ones col
            sdvec = big1.tile([128, JT, 16], F32)        # dst/src projections, all j
            srcrow_sb = consts.tile([H, RB], F32)

            sdveca = big1.tile([128, JT, H], F32)   # alpha * dst
            edv = big1.tile([128, JT, H], F32)      # exp(dst)
            edva = big1.tile([128, JT, H], F32)     # exp(alpha*dst)

            with tc.tile_pool(name="xt_pool", bufs=1) as xtp, \
                 tc.tile_pool(name="wps", bufs=2, space="PSUM") as wps:
                xlocT_sb = xtp.tile([128, KT, RB], BF)
                nc.sync.dma_start(xlocT_sb, d_xlocT.ap().rearrange("(k p) i -> p k i", p=128))
                xT_sb = xtp.tile([128, KT, N], BF)
                nc.sync.dma_start(xT_sb, d_xT.ap().rearrange("(k p) n -> p k n", p=128))

                # mask next (phase A), late weights after
                nc.sync.dma_start(a01_sb, d_a01.ap().rearrange("(t p) i -> p t i", p=128))
                wo_sb = consts.tile([128, KT, OUT], BF)
                nc.sync.dma_start(wo_sb, d_wo.ap().rearrange("(k p) o -> p k o", p=128))
                w2aux_sb = consts.tile([128, KT, 2], BF)
                nc.sync.dma_start(w2aux_sb, d_w2aux.ap().rearrange("(k p) o -> p k o", p=128))
                l1w_sb = consts.tile([128, KT, 1024], BF)
                nc.sync.dma_start(l1w_sb, d_l1w.ap().rearrange("(k p) o -> p k o", p=128))
                l1b_sb = consts.tile([1, 1024], BF)
                nc.sync.dma_start(l1b_sb, d_l1b.ap())
                l2w_sb = consts.tile([128, 8, OUT], BF)
                nc.sync.dma_start(l2w_sb, d_l2w.ap().rearrange("(k p) o -> p k o", p=128))
                l2b_sb = consts.tile([1, OUT], BF)
                nc.sync.dma_start(l2b_sb, d_l2b.ap())

                # src of local rows first (phase A's broadcast inputs gate on it)
                ps_row = wps.tile([H, RB], F32, tag="aux")
                for kt in range(KT):
                    nc.tensor.matmul(ps_row, waux_sb[:, kt, 8:16], xlocT_sb[:, kt, :],
                                     start=(kt == 0), stop=(kt == KT - 1))
                nc.vector.tensor_copy(srcrow_sb, ps_row)

                # src rows -> DRAM -> partition-broadcast tiles
                srcrow_bf = consts.tile([H, RB], BF)
                nc.vector.tensor_copy(srcrow_bf, srcrow_sb)
                nc.sync.dma_start(d_srcrow.ap(), srcrow_bf)
                esrow_sb = consts.tile([H, RB], BF)
                nc.scalar.activation(esrow_sb, srcrow_sb, AF.Exp)
                nc.sync.dma_start(d_esrow.ap(), esrow_sb)
                esarow_sb = consts.tile([H, RB], BF)
                nc.scalar.activation(esarow_sb, srcrow_sb, AF.Exp, scale=ALPHA)
                nc.sync.dma_start(d_esarow.ap(), esarow_sb)

                for jt in range(JT):
                    ps_wh = wps.tile([128, H * O], F32)
                    ps_aux = wps.tile([128, 16], F32, tag="aux")
                    for kt in range(KT):
                        lhsT = xT_sb[:, kt, jt * 128:(jt + 1) * 128]
                        nc.tensor.matmul(ps_wh, lhsT, w1_sb[:, kt, :],
                                         start=(kt == 0), stop=(kt == KT - 1))
                        nc.tensor.matmul(ps_aux, lhsT, waux_sb[:, kt, :],
                                         start=(kt == 0), stop=(kt == KT - 1))
                    nc.scalar.copy(whaug[:, jt, :, 0:O],
                                   ps_wh.rearrange("p (h o) -> p h o", h=H))
                    nc.vector.tensor_copy(sdvec[:, jt, :], ps_aux)
                    # per-jt derived scalars so phase A can start before W ends
                    nc.vector.tensor_scalar(sdveca[:, jt, :], sdvec[:, jt, 0:H],
                                            ALPHA, None, op0=AL.mult)
                    nc.scalar.activation(edv[:, jt, :], sdvec[:, jt, 0:H], AF.Exp)
                    nc.scalar.activation(edva[:, jt, :], sdvec[:, jt, 0:H], AF.Exp,
                                         scale=ALPHA)
                    nc.vector.memset(whaug[:, jt, :, O], 1.0)

            srcb = big1.tile([128, H, RB], BF)
            for h in D_HEADS + DR_HEADS:
                nc.sync.dma_start(srcb[:, h, :],
                                  d_srcrow.ap()[h:h + 1, :].to_broadcast([128, RB]))
            esb = big1.tile([128, len(R_HEADS), RB], BF)
            for k, h in enumerate(R_HEADS):
                nc.sync.dma_start(esb[:, k, :],
                                  d_esrow.ap()[h:h + 1, :].to_broadcast([128, RB]))
            esab = big1.tile([128, len(R_HEADS) + len(DR_HEADS), RB], BF)
            for k, h in enumerate(R_HEADS + DR_HEADS):
                nc.sync.dma_start(esab[:, k, :],
                                  d_esarow.ap()[h:h + 1, :].to_broadcast([128, RB]))

            # ---------------- Phase A: layer-1 attention ------------------
            # Heads run interleaved D,R,D,R,... so scalar-engine heads and
            # DVE heads overlap.  Normalization is deferred: per head only
            # num/den are staged to SBUF; one batched reciprocal + elu
            # endgame runs after the last head.
            nsball = big1.tile([128, IT, RB], F32)
            denrows = big1.tile([H, RB], F32)
            with tc.tile_pool(name="e2p", bufs=4) as e2p, \
                 tc.tile_pool(name="aps", bufs=2, space="PSUM") as aps, \
                 tc.tile_pool(name="bps", bufs=2, space="PSUM") as bps, \
                 tc.tile_pool(name="comb", bufs=2) as comb:
                for h in (0, 5, 1, 6, 2, 7, 3, 4):
                    psH = aps.tile([O + 1, RB], F32, tag="psH", name=f"psH{h}")
                    for jg in range(JT // JG):
                        a1g = e2p.tile([128, JG, RB], BF, tag="a1")
                        a2g = e2p.tile([128, JG, RB], BF, tag="a2")
                        for q in range(JG):
                            jt = jg * JG + q
                            if h in D_HEADS or h in DR_HEADS:
                                nc.scalar.activation(a1g[:, q, :], srcb[:, h, :],
                                                     AF.Exp,
                                                     bias=sdvec[:, jt, h:h + 1])
                            else:
                                k = R_HEADS.index(h)
                                nc.vector.tensor_scalar(a1g[:, q, :], esb[:, k, :],
                                                        edv[:, jt, h:h + 1], None,
                                                        op0=AL.mult)
                            if h in D_HEADS:
                                nc.scalar.activation(a2g[:, q, :], srcb[:, h, :],
                                                     AF.Exp,
                                                     bias=sdveca[:, jt, h:h + 1],
                                                     scale=ALPHA)
                            else:
                                k = (R_HEADS + DR_HEADS).index(h)
                                nc.vector.tensor_scalar(a2g[:, q, :], esab[:, k, :],
                                                        edva[:, jt, h:h + 1], None,
                                                        op0=AL.mult)
                        # batched max + adj-mask over the 4-tile group
                        e2g = e2p.tile([128, JG, RB], BF, tag="e2")
                        nc.vector.tensor_tensor(a1g, a1g, a2g, op=AL.max)
                        nc.vector.tensor_tensor(
                            e2g, a1g, a01_sb[:, jg * JG:(jg + 1) * JG, :], op=AL.mult)
                        for q in range(JG):
                            jt = jg * JG + q
                            nc.tensor.matmul(psH, whaug[:, jt, h, :], e2g[:, q, :],
                                             start=(jt == 0), stop=(jt == JT - 1))

                    # stage num/den: PSUM -> SBUF on scalar, then DMA to the
                    # head's slot (cross-partition moves need DMA); norm
                    # deferred to the endgame
                    p0 = (h % 2) * O
                    stg = comb.tile([O + 1, RB], F32, tag="stg")
                    nc.scalar.copy(stg, psH)
                    nc.sync.dma_start(nsball[p0:p0 + O, h // 2, :], stg[0:O, :])
                    nc.sync.dma_start(denrows[h:h + 1, :], stg[O:O + 1, :])

                # endgame: h1 = elu(num/den) for all heads; ONE DVE
                # reciprocal covers all 8 denominators (it is an iterative
                # 8x-cost op, so batching across partitions is essential)
                recs_f = comb.tile([H, RB], F32, tag="recsf")
                nc.vector.reciprocal(recs_f, denrows)
                recs = comb.tile([H, RB], BF, tag="recs")
                nc.vector.tensor_copy(recs, recs_f)
                nc.sync.dma_start(d_recs.ap(), recs)
                for h in range(H):
                    p0 = (h % 2) * O
                    recb = comb.tile([128, RB], BF, tag="recb")
                    nc.sync.dma_start(
                        recb[p0:p0 + O, :],
                        d_recs.ap()[h:h + 1, :].to_broadcast([O, RB]))
                    h1r = comb.tile([128, RB], F32, tag="h1r")
                    nc.vector.tensor_tensor(h1r[p0:p0 + O, :],
                                            nsball[p0:p0 + O, h // 2, :],
                                            recb[p0:p0 + O, :], op=AL.mult)
                    ex = comb.tile([128, RB], F32, tag="ex")
                    nc.scalar.activation(ex[p0:p0 + O, :], h1r[p0:p0 + O, :], AF.Exp)
                    nc.vector.tensor_scalar(ex[p0:p0 + O, :], ex[p0:p0 + O, :],
                                            1.0, -1.0, op0=AL.min, op1=AL.add)
                    nc.vector.scalar_tensor_tensor(
                        h1T[p0:p0 + O, h // 2, :], in0=h1r[p0:p0 + O, :], scalar=0.0,
                        in1=ex[p0:p0 + O, :], op0=AL.max, op1=AL.add)

            l1ctx.close()
            big2 = ctx.enter_context(tc.tile_pool(name="big2", bufs=1))

            # ---------------- Phase W2: local Wh2 + AllGather -------------
            cc2s = big2.tile([128, IT, 514], BF)   # 514 keeps 4B alignment
            with tc.tile_pool(name="w2ps", bufs=2, space="PSUM") as w2ps:
                for it in range(IT):
                    ps2 = w2ps.tile([128, OUT], F32)
                    ps2a = w2ps.tile([128, 2], F32, tag="aux2")
                    for kt in range(KT):
                        lhsT = h1T[:, kt, it * 128:(it + 1) * 128]
                        nc.tensor.matmul(ps2, lhsT, wo_sb[:, kt, :],
                                         start=(kt == 0), stop=(kt == KT - 1))
                        nc.tensor.matmul(ps2a, lhsT, w2aux_sb[:, kt, :],
                                         start=(kt == 0), stop=(kt == KT - 1))
                    nc.scalar.copy(cc2s[:, it, 0:OUT], ps2)
                    nc.vector.tensor_copy(cc2s[:, it, OUT:OUT + 1], ps2a[:, 0:1])

                # local src2 row
                ps_r2 = w2ps.tile([1, RB], F32, tag="aux2")
                for kt in range(KT):
                    nc.tensor.matmul(ps_r2, w2aux_sb[:, kt, 1:2], h1T[:, kt, :],
                                     start=(kt == 0), stop=(kt == KT - 1))
                s2row_sb = consts.tile([1, RB], BF)
                nc.vector.tensor_copy(s2row_sb, ps_r2)
                nc.sync.dma_start(d_s2row.ap(), s2row_sb)
                es2row_sb = consts.tile([1, RB], BF)
                nc.scalar.activation(es2row_sb, ps_r2, AF.Exp)
                nc.sync.dma_start(d_es2row.ap(), es2row_sb)
                esa2row_sb = consts.tile([1, RB], BF)
                nc.scalar.activation(esa2row_sb, ps_r2, AF.Exp, scale=ALPHA)
                nc.sync.dma_start(d_esa2row.ap(), esa2row_sb)

            nc.sync.dma_start(d_cc2a.ap().rearrange("(t p) c -> p t c", p=128),
                              cc2s[:, :, OUT:OUT + 1])
            nc.gpsimd.collective_compute(
                "AllGather", AL.bypass, ins=[d_cc2a.ap().opt()],
                outs=[d_cc2ao.ap().opt()], replica_groups=[list(range(NC))])
            # big gather in 2 row-chunks so A2 matmuls start on chunk 0
            for cg in range(2):
                nc.sync.dma_start(
                    d_cc2.ap()[cg * 256:(cg + 1) * 256, :]
                    .rearrange("(t p) c -> p t c", p=128),
                    cc2s[:, cg * 2:(cg + 1) * 2, 0:513])
                nc.gpsimd.collective_compute(
                    "AllGather", AL.bypass,
                    ins=[d_cc2.ap()[cg * 256:(cg + 1) * 256, :].opt()],
                    outs=[(d_cc2o0 if cg == 0 else d_cc2o1).ap().opt()],
                    replica_groups=[list(range(NC))])

            # dst2 per-partition scalars (from mini-gather) + src2 broadcast
            dst2b = big2.tile([128, JT, 1], BF)
            nc.sync.dma_start(dst2b, d_cc2ao.ap().rearrange("(t p) c -> p t c", p=128))
            dst2v = big2.tile([128, JT, 1], F32)
            nc.vector.tensor_copy(dst2v, dst2b)
            dst2va = big2.tile([128, JT, 1], F32)
            nc.vector.tensor_scalar(dst2va, dst2b, ALPHA, None, op0=AL.mult)
            src2b = big2.tile([128, RB], BF)
            nc.sync.dma_start(src2b, d_s2row.ap()[0:1, :].to_broadcast([128, RB]))
            es2b = big2.tile([128, RB], BF)
            nc.sync.dma_start(es2b, d_es2row.ap()[0:1, :].to_broadcast([128, RB]))
            esa2b = big2.tile([128, RB], BF)
            nc.sync.dma_start(esa2b, d_esa2row.ap()[0:1, :].to_broadcast([128, RB]))
            ed2v = big2.tile([128, JT, 1], F32)
            nc.scalar.activation(ed2v, dst2b, AF.Exp)
            eda2v = big2.tile([128, JT, 1], F32)
            nc.scalar.activation(eda2v, dst2b, AF.Exp, scale=ALPHA)

            # gathered Wh2 chunks: [p, chunk, (core,itm), 514]; col 512 = dst2
            # (unused), col 513 = ones (denominator row)
            wh2aug = big2.tile([128, 2, 16, 514], BF)
            nc.sync.dma_start(wh2aug[:, 0, :, 0:513],
                              d_cc2o0.ap().rearrange("(t p) c -> p t c", p=128))
            nc.sync.dma_start(wh2aug[:, 1, :, 0:513],
                              d_cc2o1.ap().rearrange("(t p) c -> p t c", p=128))
            nc.vector.memset(wh2aug[:, :, :, 513], 1.0)

            # ---------------- Phase A2: layer-2 attention (transposed) ----
            # e2 tiles for all jt are produced up-front (overlaps the gather);
            # matmuls consume them in gather-chunk order.
            o2T = big2.tile([128, IT, RB], BF)
            e2all = big2.tile([128, JT, RB], BF)
            with tc.tile_pool(name="e22p", bufs=2) as e22p, \
                 tc.tile_pool(name="a2ps", bufs=1, space="PSUM") as a2ps, \
                 tc.tile_pool(name="b2ps", bufs=2, space="PSUM") as b2ps, \
                 tc.tile_pool(name="c2", bufs=2) as c2:
                for jg in range(JT // JG):
                    a1g = e22p.tile([128, JG, RB], BF, tag="a1")
                    a2g = e22p.tile([128, JG, RB], BF, tag="a2")
                    for q in range(JG):
                        jt = jg * JG + q
                        if jg % 2 == 0:   # scalar-engine form
                            nc.scalar.activation(a1g[:, q, :], src2b, AF.Exp,
                                                 bias=dst2v[:, jt, 0:1])
                            nc.scalar.activation(a2g[:, q, :], src2b, AF.Exp,
                                                 bias=dst2va[:, jt, 0:1], scale=ALPHA)
                        else:             # DVE rank-1 form
                            nc.vector.tensor_scalar(a1g[:, q, :], es2b,
                                                    ed2v[:, jt, 0:1], None,
                                                    op0=AL.mult)
                            nc.vector.tensor_scalar(a2g[:, q, :], esa2b,
                                                    eda2v[:, jt, 0:1], None,
                                                    op0=AL.mult)
                    nc.vector.tensor_tensor(a1g, a1g, a2g, op=AL.max)
                    nc.vector.tensor_tensor(
                        e2all[:, jg * JG:(jg + 1) * JG, :], a1g,
                        a01_sb[:, jg * JG:(jg + 1) * JG, :], op=AL.mult)

                psO = [a2ps.tile([128, RB], F32, tag=f"psO{c}", name=f"psO{c}")
                       for c in range(IT)]
                psD = a2ps.tile([1, RB], F32, tag="psD", name="psD")
                for cg in range(2):
                    for u in range(16):
                        jt = (u // 2) * 4 + cg * 2 + (u % 2)
                        st = (cg == 0 and u == 0)
                        sp = (cg == 1 and u == 15)
                        e2 = e2all[:, jt, :]
                        for c in range(IT):
                            nc.tensor.matmul(psO[c],
                                             wh2aug[:, cg, u, c * 128:(c + 1) * 128],
                                             e2, start=st, stop=sp)
                        nc.tensor.matmul(psD, wh2aug[:, cg, u, 513:514], e2,
                                         start=st, stop=sp)

                lden2 = c2.tile([1, RB], F32, tag="lden2")
                nc.scalar.activation(lden2, psD, AF.Ln)
                rec2 = c2.tile([1, RB], BF, tag="rec2")
                nc.scalar.activation(rec2, lden2, AF.Exp, scale=-1.0)
                psB2 = b2ps.tile([128, RB], F32)
                ones128 = consts.tile([1, 128], BF)
                nc.vector.memset(ones128, 1.0)
                nc.tensor.matmul(psB2, ones128, rec2)
                for c in range(IT):
                    nsb2 = c2.tile([128, RB], F32, tag="nsb2")
                    nc.scalar.copy(nsb2, psO[c])
                    o2r = c2.tile([128, RB], F32, tag="o2r")
                    nc.vector.tensor_tensor(o2r, nsb2, psB2, op=AL.mult)
                    ex2 = c2.tile([128, RB], F32, tag="ex2")
                    nc.scalar.activation(ex2, o2r, AF.Exp)
                    nc.vector.tensor_scalar(ex2, ex2, 1.0, -1.0, op0=AL.min, op1=AL.add)
                    nc.vector.scalar_tensor_tensor(o2T[:, c, :], in0=o2r, scalar=0.0,
                                                   in1=ex2, op0=AL.max, op1=AL.add)

            # ---------------- lin1 + lin2 ---------------------------------
            o3T = big2.tile([128, 8, RB], BF)
            out_sb = big2.tile([128, IT, OUT], F32)
            with tc.tile_pool(name="l_ps", bufs=4, space="PSUM") as lps:
                for mq in range(8):
                    ps = lps.tile([128, RB], F32)
                    for kt in range(KT):
                        nc.tensor.matmul(ps, l1w_sb[:, kt, mq * 128:(mq + 1) * 128],
                                         o2T[:, kt, :], start=(kt == 0), stop=False)
                    nc.tensor.matmul(ps, l1b_sb[:, mq * 128:(mq + 1) * 128], ones_row,
                                     start=False, stop=True)
                    nc.scalar.activation(o3T[:, mq, :], ps, AF.Relu)

                for mi in range(IT):
                    ps = lps.tile([128, OUT], F32)
                    for kq in range(8):
                        nc.tensor.matmul(ps, o3T[:, kq, mi * 128:(mi + 1) * 128],
                                         l2w_sb[:, kq, :], start=(kq == 0), stop=False)
                    nc.tensor.matmul(ps, ones_row[:, 0:128], l2b_sb,
                                     start=False, stop=True)
                    nc.vector.tensor_copy(out_sb[:, mi, :], ps)

            nc.sync.dma_start(d_out.ap().rearrange("(t p) o -> p t o", p=128), out_sb)

    nc.compile()
    return nc


_CACHE = {}


def _prep_inputs(inputs):
    x = np.asarray(inputs["x"], np.float32)
    adj = np.asarray(inputs["adj"])
    W1 = np.asarray(inputs["W1"], np.float32)
    a1 = np.asarray(inputs["a1"], np.float32)
    Wo = np.asarray(inputs["Wo"], np.float32)
    ao = np.asarray(inputs["ao"], np.float32)
    l1w = np.asarray(inputs["lin1_w"], np.float32)
    l1b = np.asarray(inputs["lin1_b"], np.float32)
    l2w = np.asarray(inputs["lin2_w"], np.float32)
    l2b = np.asarray(inputs["lin2_b"], np.float32)

    xT = np.ascontiguousarray(x.T).astype(BF16)
    w_dst = np.einsum("hfo,ho->fh", W1, a1[:, O:]).astype(np.float32)   # [F, H]
    w_src = np.einsum("hfo,ho->fh", W1, a1[:, :O]).astype(np.float32)
    wauxall = np.ascontiguousarray(
        np.concatenate([w_dst, w_src], axis=1)).astype(BF16)            # [F, 16]
    w1all = np.ascontiguousarray(W1.transpose(1, 0, 2).reshape(FEAT, H * O)).astype(BF16)
    w2aux = np.ascontiguousarray(
        np.stack([Wo @ ao[OUT:], Wo @ ao[:OUT]], axis=1)).astype(BF16)  # [F, 2]

    rep = dict(
        xT=xT, w1all=w1all, wauxall=wauxall, woall=np.ascontiguousarray(Wo).astype(BF16),
        w2aux=w2aux,
        l1w=np.ascontiguousarray(l1w).astype(BF16),
        l1b=np.ascontiguousarray(l1b.reshape(1, -1)).astype(BF16),
        l2w=np.ascontiguousarray(l2w).astype(BF16),
        l2b=np.ascontiguousarray(l2b.reshape(1, -1)).astype(BF16),
    )
    in_maps = []
    for c in range(NC):
        rows = slice(c * RB, (c + 1) * RB)
        m = dict(rep)
        m["xlocT"] = np.ascontiguousarray(x[rows, :].T).astype(BF16)
        m["a01T"] = np.ascontiguousarray((adj[rows, :] > 0).T.astype(BF16))
        in_maps.append(m)
    return in_maps


def kernel(**inputs):
    from concourse.bass_utils import run_bass_kernel_spmd

    if "nc" not in _CACHE:
        _CACHE["nc"] = build_program()
    nc = _CACHE["nc"]

    in_maps = _prep_inputs(inputs)
    trace = bool(_CACHE.get("trace"))
    res = run_bass_kernel_spmd(nc, in_maps, core_ids=list(range(NC)), trace=trace)
    _CACHE["last_results"] = res
    out = np.concatenate([r["out"] for r in res.results], axis=0)
    return out.astype(np.float32)


# revision 42
# speedup vs baseline: 1.4181x; 1.1784x over previous
# Self-contained Trainium2 Bass kernel for the 2-layer dense GAT problem.
# kernel(**inputs) takes FULL inputs, shards rows across 8 NeuronCores,
# runs one SPMD Bass program (with mid-kernel AllGathers), returns FULL output.
#
# Layer-1/2 attention strategy: exp(leakyrelu(s)) == max(exp(s), exp(a*s))
# exactly for a<=1, and exp(src_i+dst_j) factors rank-1.  So the masked
# attention matrix e2 = adj * max(exp(s), exp(a*s)) is built per [128j,512i]
# tile with two scalar-engine Exp activations (per-partition dst bias) or two
# DVE tensor_scalar rank-1 products, a max and an adj-mask multiply, spread
# across Scalar/Vector/GpSimd.  Aggregation is a single transposed matmul
# (stationary = Whaug[j,65]) per tile, accumulating [65,512] in PSUM, giving
# h1T [feat, i] directly (no transpose phase).
import os
import sys

os.environ.setdefault("JAX_PLATFORMS", "axon")
sys.path.insert(0, "/opt/trn_rl_repo")

import numpy as np
import ml_dtypes

import concourse.bass as bass
import concourse.bacc as bacc
import concourse.tile as tile
from concourse import mybir

BF16 = ml_dtypes.bfloat16
F32 = mybir.dt.float32
BF = mybir.dt.bfloat16

N, FEAT, H, O, OUT = 4096, 512, 8, 64, 512
NC = 8
RB = N // NC          # 512 rows per core
JT = N // 128         # 32 j tiles
KT = FEAT // 128      # 4 feature tiles
IT = RB // 128        # 4 local row tiles
ALPHA = 0.2

AL = mybir.AluOpType
AF = mybir.ActivationFunctionType

# head formulation mix: even heads build exps on the scalar engine, odd
# heads as DVE rank-1 products; one even + one odd head run concurrently
# so both engines stay busy, and h1T feature blocks finish in order.
D_HEADS = (0, 2, 4, 6)
R_HEADS = (1, 3, 5, 7)
DR_HEADS = ()
JG = 4                        # j-tiles per batched DVE max/mask op


def _bc(ap, n):
    """Broadcast an AP along a new innermost free dim of size n (step 0)."""
    return bass.AP(tensor=ap.tensor, offset=ap.offset, ap=list(ap.ap) + [[0, n]])


def build_program():
    nc = bacc.Bacc("TRN2", target_bir_lowering=False, debug=False, num_devices=NC)

    # ---------------- DRAM I/O ----------------
    d_xT = nc.dram_tensor("xT", [FEAT, N], BF, kind="ExternalInput")          # replicated
    d_xlocT = nc.dram_tensor("xlocT", [FEAT, RB], BF, kind="ExternalInput")   # per-core
    d_a01 = nc.dram_tensor("a01T", [N, RB], BF, kind="ExternalInput")         # per-core
    d_w1 = nc.dram_tensor("w1all", [FEAT, H * O], BF, kind="ExternalInput")
    d_waux = nc.dram_tensor("wauxall", [FEAT, 16], BF, kind="ExternalInput")  # [dst8|src8]
    d_wo = nc.dram_tensor("woall", [FEAT, OUT], BF, kind="ExternalInput")
    d_w2aux = nc.dram_tensor("w2aux", [FEAT, 2], BF, kind="ExternalInput")    # [dst|src]
    d_l1w = nc.dram_tensor("l1w", [OUT, 1024], BF, kind="ExternalInput")
    d_l1b = nc.dram_tensor("l1b", [1, 1024], BF, kind="ExternalInput")
    d_l2w = nc.dram_tensor("l2w", [1024, OUT], BF, kind="ExternalInput")
    d_l2b = nc.dram_tensor("l2b", [1, OUT], BF, kind="ExternalInput")
    d_out = nc.dram_tensor("out", [RB, OUT], F32, kind="ExternalOutput")

    # internal DRAM scratch
    d_srcrow = nc.dram_tensor("srcrow_d", [H, RB], BF)
    d_recs = nc.dram_tensor("recs_d", [H, RB], BF)
    d_esrow = nc.dram_tensor("esrow_d", [H, RB], BF)
    d_esarow = nc.dram_tensor("esarow_d", [H, RB], BF)
    d_s2row = nc.dram_tensor("s2row_d", [1, RB], BF)
    d_es2row = nc.dram_tensor("es2row_d", [1, RB], BF)
    d_esa2row = nc.dram_tensor("esa2row_d", [1, RB], BF)
    d_cc2 = nc.dram_tensor("cc2_d", [RB, 513], BF)
    d_cc2o0 = nc.dram_tensor("cc2o0_d", [N // 2, 513], BF, addr_space="Shared")
    d_cc2o1 = nc.dram_tensor("cc2o1_d", [N // 2, 513], BF, addr_space="Shared")

    with tile.TileContext(nc) as tc:
        import contextlib
        ctx = contextlib.ExitStack()
        with ctx:
            consts = ctx.enter_context(tc.tile_pool(name="consts", bufs=1))

            ones_row = consts.tile([1, RB], BF)
            nc.vector.memset(ones_row, 1.0)
            ones64 = consts.tile([1, O], BF)
            nc.vector.memset(ones64, 1.0)

            # ---------------- weights into SBUF (phase-W inputs first) ----
            w1_sb = consts.tile([128, KT, H * O], BF)
            nc.sync.dma_start(w1_sb, d_w1.ap().rearrange("(k p) o -> p k o", p=128))
            waux_sb = consts.tile([128, KT, 16], BF)
            nc.sync.dma_start(waux_sb, d_waux.ap().rearrange("(k p) o -> p k o", p=128))

            p_a01 = ctx.enter_context(tc.tile_pool(name="p_a01", bufs=1))
            a01_sb = p_a01.tile([128, JT, RB], BF)
            p_h1 = ctx.enter_context(tc.tile_pool(name="p_h1", bufs=1))
            h1T = p_h1.tile([128, IT, RB], BF)   # layer-1 out, [feat, local-row]

            import contextlib as _cl
            l1ctx = _cl.ExitStack()
            big1 = l1ctx.enter_context(tc.tile_pool(name="big1", bufs=1))

            # ---------------- Phase W: Wh1 + aux projections (full N) -----
            whaug = big1.tile([128, JT, H, O + 1], BF)   # per-j Wh per head + ones col
            sdvec = big1.tile([128, JT, 16], F32)        # dst/src projections, all j
            srcrow_sb = consts.tile([H, RB], F32)

            sdveca = big1.tile([128, JT, H], F32)   # alpha * dst
            edv = big1.tile([128, JT, H], F32)      # exp(dst)
            edva = big1.tile([128, JT, H], F32)     # exp(alpha*dst)

            with tc.tile_pool(name="xt_pool", bufs=1) as xtp, \
                 tc.tile_pool(name="wps", bufs=2, space="PSUM") as wps:
                xlocT_sb = xtp.tile([128, KT, RB], BF)
                nc.sync.dma_start(xlocT_sb, d_xlocT.ap().rearrange("(k p) i -> p k i", p=128))
                xT_sb = xtp.tile([128, KT, N], BF)
                nc.sync.dma_start(xT_sb, d_xT.ap().rearrange("(k p) n -> p k n", p=128))

                # mask next (phase A), late weights after
                nc.sync.dma_start(a01_sb, d_a01.ap().rearrange("(t p) i -> p t i", p=128))
                wo_sb = consts.tile([128, KT, OUT], BF)
                nc.sync.dma_start(wo_sb, d_wo.ap().rearrange("(k p) o -> p k o", p=128))
                w2aux_sb = consts.tile([128, KT, 2], BF)
                nc.sync.dma_start(w2aux_sb, d_w2aux.ap().rearrange("(k p) o -> p k o", p=128))
                l1w_sb = consts.tile([128, KT, 1024], BF)
                nc.sync.dma_start(l1w_sb, d_l1w.ap().rearrange("(k p) o -> p k o", p=128))
                l1b_sb = consts.tile([1, 1024], BF)
                nc.sync.dma_start(l1b_sb, d_l1b.ap())
                l2w_sb = consts.tile([128, 8, OUT], BF)
                nc.sync.dma_start(l2w_sb, d_l2w.ap().rearrange("(k p) o -> p k o", p=128))
                l2b_sb = consts.tile([1, OUT], BF)
                nc.sync.dma_start(l2b_sb, d_l2b.ap())

                # src of local rows first (phase A's broadcast inputs gate on it)
                ps_row = wps.tile([H, RB], F32, tag="aux")
                for kt in range(KT):
                    nc.tensor.matmul(ps_row, waux_sb[:, kt, 8:16], xlocT_sb[:, kt, :],
                                     start=(kt == 0), stop=(kt == KT - 1))
                nc.vector.tensor_copy(srcrow_sb, ps_row)

                # src rows -> DRAM -> partition-broadcast tiles
                srcrow_bf = consts.tile([H, RB], BF)
                nc.vector.tensor_copy(srcrow_bf, srcrow_sb)
                nc.sync.dma_start(d_srcrow.ap(), srcrow_bf)
                esrow_sb = consts.tile([H, RB], BF)
                nc.scalar.activation(esrow_sb, srcrow_sb, AF.Exp)
                nc.sync.dma_start(d_esrow.ap(), esrow_sb)
                esarow_sb = consts.tile([H, RB], BF)
                nc.scalar.activation(esarow_sb, srcrow_sb, AF.Exp, scale=ALPHA)
                nc.sync.dma_start(d_esarow.ap(), esarow_sb)

                for jt in range(JT):
                    ps_wh = wps.tile([128, H * O], F32)
                    ps_aux = wps.tile([128, 16], F32, tag="aux")
                    for kt in range(KT):
                        lhsT = xT_sb[:, kt, jt * 128:(jt + 1) * 128]
                        nc.tensor.matmul(ps_wh, lhsT, w1_sb[:, kt, :],
                                         start=(kt == 0), stop=(kt == KT - 1))
                        nc.tensor.matmul(ps_aux, lhsT, waux_sb[:, kt, :],
                                         start=(kt == 0), stop=(kt == KT - 1))
                    nc.vector.tensor_copy(whaug[:, jt, :, 0:O],
                                          ps_wh.rearrange("p (h o) -> p h o", h=H))
                    nc.vector.tensor_copy(sdvec[:, jt, :], ps_aux)
                    # per-jt derived scalars so phase A can start before W ends
                    nc.vector.tensor_scalar(sdveca[:, jt, :], sdvec[:, jt, 0:H],
                                            ALPHA, None, op0=AL.mult)
                    nc.scalar.activation(edv[:, jt, :], sdvec[:, jt, 0:H], AF.Exp)
                    nc.scalar.activation(edva[:, jt, :], sdvec[:, jt, 0:H], AF.Exp,
                                         scale=ALPHA)
                    nc.vector.memset(whaug[:, jt, :, O], 1.0)

            srcb = big1.tile([128, H, RB], BF)
            for h in D_HEADS + DR_HEADS:
                nc.sync.dma_start(srcb[:, h, :],
                                  d_srcrow.ap()[h:h + 1, :].to_broadcast([128, RB]))
            esb = big1.tile([128, len(R_HEADS), RB], BF)
            for k, h in enumerate(R_HEADS):
                nc.sync.dma_start(esb[:, k, :],
                                  d_esrow.ap()[h:h + 1, :].to_broadcast([128, RB]))
            esab = big1.tile([128, len(R_HEADS) + len(DR_HEADS), RB], BF)
            for k, h in enumerate(R_HEADS + DR_HEADS):
                nc.sync.dma_start(esab[:, k, :],
                                  d_esarow.ap()[h:h + 1, :].to_broadcast([128, RB]))

            # ---------------- Phase A: layer-1 attention ------------------
            # Heads run interleaved D,R,D,R,... so scalar-engine heads and
            # DVE heads overlap.  Normalization is deferred: per head only
            # num/den are staged to SBUF; one batched reciprocal + elu
            # endgame runs after the last head.
            nsball = big1.tile([128, IT, RB], F32)
            denrows = big1.tile([H, RB], F32)
            with tc.tile_pool(name="e2p", bufs=3) as e2p, \
                 tc.tile_pool(name="aps", bufs=2, space="PSUM") as aps, \
                 tc.tile_pool(name="comb", bufs=2) as comb:
                for pr in range(H // 2):
                    hd, hr = 2 * pr, 2 * pr + 1
                    kr = R_HEADS.index(hr)
                    psHd = aps.tile([O + 1, RB], F32, tag="psHd", name=f"psHd{hd}")
                    psHr = aps.tile([O + 1, RB], F32, tag="psHr", name=f"psHr{hr}")
                    for jg in range(JT // JG):
                        j0 = jg * JG
                        a1d = e2p.tile([128, JG, RB], BF, tag="a1d")
                        a2d = e2p.tile([128, JG, RB], BF, tag="a2d")
                        a1r = e2p.tile([128, JG, RB], BF, tag="a1r")
                        a2r = e2p.tile([128, JG, RB], BF, tag="a2r")
                        for q in range(JG):
                            jt = j0 + q
                            nc.scalar.activation(a1d[:, q, :], srcb[:, hd, :],
                                                 AF.Exp, bias=sdvec[:, jt, hd:hd + 1])
                            nc.scalar.activation(a2d[:, q, :], srcb[:, hd, :],
                                                 AF.Exp, bias=sdveca[:, jt, hd:hd + 1],
                                                 scale=ALPHA)
                            nc.vector.tensor_scalar(a1r[:, q, :], esb[:, kr, :],
                                                    edv[:, jt, hr:hr + 1], None,
                                                    op0=AL.mult)
                            nc.vector.tensor_scalar(a2r[:, q, :], esab[:, kr, :],
                                                    edva[:, jt, hr:hr + 1], None,
                                                    op0=AL.mult)
                        # batched max; adj-mask written in place of the a2 exps
                        a01g = a01_sb[:, j0:j0 + JG, :]
                        nc.vector.tensor_tensor(a1d, a1d, a2d, op=AL.max)
                        nc.vector.tensor_tensor(a2d, a1d, a01g, op=AL.mult)
                        nc.vector.tensor_tensor(a1r, a1r, a2r, op=AL.max)
                        nc.vector.tensor_tensor(a2r, a1r, a01g, op=AL.mult)
                        for q in range(JG):
                            jt = j0 + q
                            st, sp = (jt == 0), (jt == JT - 1)
                            nc.tensor.matmul(psHd, whaug[:, jt, hd, :], a2d[:, q, :],
                                             start=st, stop=sp)
                            nc.tensor.matmul(psHr, whaug[:, jt, hr, :], a2r[:, q, :],
                                             start=st, stop=sp)

                    # stage num/den: PSUM -> SBUF on scalar, then DMA to the
                    # head's slot (cross-partition moves need DMA); norm
                    # deferred to the endgame
                    for h, psH in ((hd, psHd), (hr, psHr)):
                        p0 = (h % 2) * O
                        stg = comb.tile([O + 1, RB], F32, tag="stg")
                        nc.scalar.copy(stg, psH)
                        nc.sync.dma_start(nsball[p0:p0 + O, h // 2, :], stg[0:O, :])
                        nc.sync.dma_start(denrows[h:h + 1, :], stg[O:O + 1, :])

                # endgame: h1 = elu(num/den) for all heads; ONE DVE
                # reciprocal covers all 8 denominators (it is an iterative
                # 8x-cost op, so batching across partitions is essential)
                recs_f = comb.tile([H, RB], F32, tag="recsf")
                nc.vector.reciprocal(recs_f, denrows)
                recs = comb.tile([H, RB], BF, tag="recs")
                nc.vector.tensor_copy(recs, recs_f)
                nc.sync.dma_start(d_recs.ap(), recs)
                for h in range(H):
                    p0 = (h % 2) * O
                    recb = comb.tile([128, RB], BF, tag="recb")
                    nc.sync.dma_start(
                        recb[p0:p0 + O, :],
                        d_recs.ap()[h:h + 1, :].to_broadcast([O, RB]))
                    h1r = comb.tile([128, RB], F32, tag="h1r")
                    nc.vector.tensor_tensor(h1r[p0:p0 + O, :],
                                            nsball[p0:p0 + O, h // 2, :],
                                            recb[p0:p0 + O, :], op=AL.mult)
                    ex = comb.tile([128, RB], F32, tag="ex")
                    nc.scalar.activation(ex[p0:p0 + O, :], h1r[p0:p0 + O, :], AF.Exp)
                    nc.vector.tensor_scalar(ex[p0:p0 + O, :], ex[p0:p0 + O, :],
                                            1.0, -1.0, op0=AL.min, op1=AL.add)
                    nc.vector.scalar_tensor_tensor(
                        h1T[p0:p0 + O, h // 2, :], in0=h1r[p0:p0 + O, :], scalar=0.0,
                        in1=ex[p0:p0 + O, :], op0=AL.max, op1=AL.add)

            l1ctx.close()
            big2 = ctx.enter_context(tc.tile_pool(name="big2", bufs=1))

            # ---------------- Phase W2: local Wh2 + AllGather -------------
            cc2s = big2.tile([128, IT, 514], BF)   # 514 keeps 4B alignment
            with tc.tile_pool(name="w2ps", bufs=2, space="PSUM") as w2ps:
                for it in range(IT):
                    ps2 = w2ps.tile([128, OUT], F32)
                    ps2a = w2ps.tile([128, 2], F32, tag="aux2")
                    for kt in range(KT):
                        lhsT = h1T[:, kt, it * 128:(it + 1) * 128]
                        nc.tensor.matmul(ps2, lhsT, wo_sb[:, kt, :],
                                         start=(kt == 0), stop=(kt == KT - 1))
                        nc.tensor.matmul(ps2a, lhsT, w2aux_sb[:, kt, :],
                                         start=(kt == 0), stop=(kt == KT - 1))
                    nc.scalar.copy(cc2s[:, it, 0:OUT], ps2)
                    nc.vector.tensor_copy(cc2s[:, it, OUT:OUT + 1], ps2a[:, 0:1])

                # local src2 row
                ps_r2 = w2ps.tile([1, RB], F32, tag="aux2")
                for kt in range(KT):
                    nc.tensor.matmul(ps_r2, w2aux_sb[:, kt, 1:2], h1T[:, kt, :],
                                     start=(kt == 0), stop=(kt == KT - 1))
                s2row_sb = consts.tile([1, RB], BF)
                nc.vector.tensor_copy(s2row_sb, ps_r2)
                nc.sync.dma_start(d_s2row.ap(), s2row_sb)
                es2row_sb = consts.tile([1, RB], BF)
                nc.scalar.activation(es2row_sb, ps_r2, AF.Exp)
                nc.sync.dma_start(d_es2row.ap(), es2row_sb)
                esa2row_sb = consts.tile([1, RB], BF)
                nc.scalar.activation(esa2row_sb, ps_r2, AF.Exp, scale=ALPHA)
                nc.sync.dma_start(d_esa2row.ap(), esa2row_sb)

            # big gather in 2 row-chunks so A2 starts on chunk 0; dst2 for
            # each chunk's rows rides in column 512 of the gathered data
            for cg in range(2):
                nc.sync.dma_start(
                    d_cc2.ap()[cg * 256:(cg + 1) * 256, :]
                    .rearrange("(t p) c -> p t c", p=128),
                    cc2s[:, cg * 2:(cg + 1) * 2, 0:513])
                nc.gpsimd.collective_compute(
                    "AllGather", AL.bypass,
                    ins=[d_cc2.ap()[cg * 256:(cg + 1) * 256, :].opt()],
                    outs=[(d_cc2o0 if cg == 0 else d_cc2o1).ap().opt()],
                    replica_groups=[list(range(NC))])

            src2b = big2.tile([128, RB], BF)
            nc.sync.dma_start(src2b, d_s2row.ap()[0:1, :].to_broadcast([128, RB]))
            es2b = big2.tile([128, RB], BF)
            nc.sync.dma_start(es2b, d_es2row.ap()[0:1, :].to_broadcast([128, RB]))
            esa2b = big2.tile([128, RB], BF)
            nc.sync.dma_start(esa2b, d_esa2row.ap()[0:1, :].to_broadcast([128, RB]))

            # gathered Wh2 chunks: [p, chunk, (core,itm), 514]; col 512 =
            # dst2 of that row, col 513 = ones (denominator row)
            wh2aug = big2.tile([128, 2, 16, 514], BF)
            nc.sync.dma_start(wh2aug[:, 0, :, 0:513],
                              d_cc2o0.ap().rearrange("(t p) c -> p t c", p=128))
            nc.sync.dma_start(wh2aug[:, 1, :, 0:513],
                              d_cc2o1.ap().rearrange("(t p) c -> p t c", p=128))
            nc.vector.memset(wh2aug[:, :, :, 513], 1.0)

            # ---------------- Phase A2: layer-2 attention (transposed) ----
            # fully chunk-driven: as soon as a gather chunk lands, its dst2
            # scalars, e2 tiles and matmuls run; chunk 1 overlaps chunk 0's
            # consumption
            o2T = big2.tile([128, IT, RB], BF)
            with tc.tile_pool(name="e22p", bufs=3) as e22p, \
                 tc.tile_pool(name="a2ps", bufs=1, space="PSUM") as a2ps, \
                 tc.tile_pool(name="b2ps", bufs=2, space="PSUM") as b2ps, \
                 tc.tile_pool(name="c2", bufs=2) as c2:
                psO = [a2ps.tile([128, RB], F32, tag=f"psO{c}", name=f"psO{c}")
                       for c in range(IT)]
                psD = a2ps.tile([1, RB], F32, tag="psD", name="psD")
                dst2v = big2.tile([128, 2, 16], F32)
                dst2va = big2.tile([128, 2, 16], F32)
                ed2v = big2.tile([128, 2, 16], F32)
                eda2v = big2.tile([128, 2, 16], F32)
                for cg in range(2):
                    nc.vector.tensor_copy(dst2v[:, cg, :], wh2aug[:, cg, :, 512])
                    nc.vector.tensor_scalar(dst2va[:, cg, :], dst2v[:, cg, :],
                                            ALPHA, None, op0=AL.mult)
                    nc.scalar.activation(ed2v[:, cg, :], dst2v[:, cg, :], AF.Exp)
                    nc.scalar.activation(eda2v[:, cg, :], dst2v[:, cg, :], AF.Exp,
                                         scale=ALPHA)
                    for ug in range(4):
                        a1g = e22p.tile([128, 4, RB], BF, tag="a1")
                        a2g = e22p.tile([128, 4, RB], BF, tag="a2")
                        for q in range(4):
                            u = ug * 4 + q
                            if ug % 2 == 0:   # scalar-engine form
                                nc.scalar.activation(a1g[:, q, :], src2b, AF.Exp,
                                                     bias=dst2v[:, cg, u:u + 1])
                                nc.scalar.activation(a2g[:, q, :], src2b, AF.Exp,
                                                     bias=dst2va[:, cg, u:u + 1],
                                                     scale=ALPHA)
                            else:             # DVE rank-1 form
                                nc.vector.tensor_scalar(a1g[:, q, :], es2b,
                                                        ed2v[:, cg, u:u + 1], None,
                                                        op0=AL.mult)
                                nc.vector.tensor_scalar(a2g[:, q, :], esa2b,
                                                        eda2v[:, cg, u:u + 1], None,
                                                        op0=AL.mult)
                        nc.vector.tensor_tensor(a1g, a1g, a2g, op=AL.max)
                        for q in range(4):
                            u = ug * 4 + q
                            jt = (u // 2) * 4 + cg * 2 + (u % 2)
                            nc.vector.tensor_tensor(a2g[:, q, :], a1g[:, q, :],
                                                    a01_sb[:, jt, :], op=AL.mult)
                            st = (cg == 0 and u == 0)
                            sp = (cg == 1 and u == 15)
                            for c in range(IT):
                                nc.tensor.matmul(
                                    psO[c], wh2aug[:, cg, u, c * 128:(c + 1) * 128],
                                    a2g[:, q, :], start=st, stop=sp)
                            nc.tensor.matmul(psD, wh2aug[:, cg, u, 513:514],
                                             a2g[:, q, :], start=st, stop=sp)

                lden2 = c2.tile([1, RB], F32, tag="lden2")
                nc.scalar.activation(lden2, psD, AF.Ln)
                rec2 = c2.tile([1, RB], BF, tag="rec2")
                nc.scalar.activation(rec2, lden2, AF.Exp, scale=-1.0)
                psB2 = b2ps.tile([128, RB], F32)
                ones128 = consts.tile([1, 128], BF)
                nc.vector.memset(ones128, 1.0)
                nc.tensor.matmul(psB2, ones128, rec2)
                for c in range(IT):
                    nsb2 = c2.tile([128, RB], F32, tag="nsb2")
                    nc.scalar.copy(nsb2, psO[c])
                    o2r = c2.tile([128, RB], F32, tag="o2r")
                    nc.vector.tensor_tensor(o2r, nsb2, psB2, op=AL.mult)
                    ex2 = c2.tile([128, RB], F32, tag="ex2")
                    nc.scalar.activation(ex2, o2r, AF.Exp)
                    nc.vector.tensor_scalar(ex2, ex2, 1.0, -1.0, op0=AL.min, op1=AL.add)
                    nc.vector.scalar_tensor_tensor(o2T[:, c, :], in0=o2r, scalar=0.0,
                                                   in1=ex2, op0=AL.max, op1=AL.add)

            # ---------------- lin1 + lin2 ---------------------------------
            o3T = big2.tile([128, 8, RB], BF)
            out_sb = big2.tile([128, IT, OUT], F32)
            with tc.tile_pool(name="l_ps", bufs=4, space="PSUM") as lps:
                for mq in range(8):
                    ps = lps.tile([128, RB], F32)
                    for kt in range(KT):
                        nc.tensor.matmul(ps, l1w_sb[:, kt, mq * 128:(mq + 1) * 128],
                                         o2T[:, kt, :], start=(kt == 0), stop=False)
                    nc.tensor.matmul(ps, l1b_sb[:, mq * 128:(mq + 1) * 128], ones_row,
                                     start=False, stop=True)
                    nc.scalar.activation(o3T[:, mq, :], ps, AF.Relu)

                for mi in range(IT):
                    ps = lps.tile([128, OUT], F32)
                    for kq in range(8):
                        nc.tensor.matmul(ps, o3T[:, kq, mi * 128:(mi + 1) * 128],
                                         l2w_sb[:, kq, :], start=(kq == 0), stop=False)
                    nc.tensor.matmul(ps, ones_row[:, 0:128], l2b_sb,
                                     start=False, stop=True)
                    nc.vector.tensor_copy(out_sb[:, mi, :], ps)

            nc.sync.dma_start(d_out.ap().rearrange("(t p) o -> p t o", p=128), out_sb)

    nc.compile()
    return nc


_CACHE = {}


def _prep_inputs(inputs):
    x = np.asarray(inputs["x"], np.float32)
    adj = np.asarray(inputs["adj"])
    W1 = np.asarray(inputs["W1"], np.float32)
    a1 = np.asarray(inputs["a1"], np.float32)
    Wo = np.asarray(inputs["Wo"], np.float32)
    ao = np.asarray(inputs["ao"], np.float32)
    l1w = np.asarray(inputs["lin1_w"], np.float32)
    l1b = np.asarray(inputs["lin1_b"], np.float32)
    l2w = np.asarray(inputs["lin2_w"], np.float32)
    l2b = np.asarray(inputs["lin2_b"], np.float32)

    xT = np.ascontiguousarray(x.T).astype(BF16)
    w_dst = np.einsum("hfo,ho->fh", W1, a1[:, O:]).astype(np.float32)   # [F, H]
    w_src = np.einsum("hfo,ho->fh", W1, a1[:, :O]).astype(np.float32)
    wauxall = np.ascontiguousarray(
        np.concatenate([w_dst, w_src], axis=1)).astype(BF16)            # [F, 16]
    w1all = np.ascontiguousarray(W1.transpose(1, 0, 2).reshape(FEAT, H * O)).astype(BF16)
    w2aux = np.ascontiguousarray(
        np.stack([Wo @ ao[OUT:], Wo @ ao[:OUT]], axis=1)).astype(BF16)  # [F, 2]

    rep = dict(
        xT=xT, w1all=w1all, wauxall=wauxall, woall=np.ascontiguousarray(Wo).astype(BF16),
        w2aux=w2aux,
        l1w=np.ascontiguousarray(l1w).astype(BF16),
        l1b=np.ascontiguousarray(l1b.reshape(1, -1)).astype(BF16),
        l2w=np.ascontiguousarray(l2w).astype(BF16),
        l2b=np.ascontiguousarray(l2b.reshape(1, -1)).astype(BF16),
    )
    in_maps = []
    for c in range(NC):
        rows = slice(c * RB, (c + 1) * RB)
        m = dict(rep)
        m["xlocT"] = np.ascontiguousarray(x[rows, :].T).astype(BF16)
        m["a01T"] = np.ascontiguousarray((adj[rows, :] > 0).T.astype(BF16))
        in_maps.append(m)
    return in_maps


def kernel(**inputs):
    from concourse.bass_utils import run_bass_kernel_spmd

    if "nc" not in _CACHE:
        _CACHE["nc"] = build_program()
    nc = _CACHE["nc"]

    in_maps = _prep_inputs(inputs)
    trace = bool(_CACHE.get("trace"))
    res = run_bass_kernel_spmd(nc, in_maps, core_ids=list(range(NC)), trace=trace)
    _CACHE["last_results"] = res
    out = np.concatenate([r["out"] for r in res.results], axis=0)
    return out.astype(np.float32)
